# revision 16
# baseline (speedup 1.0000x reference)
"""Trainium2 Bass kernel for nn_PoetryModel (dense transformer, B=4 T=1024 E=512
H=8 HD=64 NB=5 VOCAB=32000), 8 NeuronCores.

Sharding: core c = 2*b + jv handles batch b = c//2 and vocab half jv = c%2.
Each core runs the full 5-block transformer for its sequence (block compute is
duplicated across the pair -- no cross-core communication needed), then projects
its sequence onto its 16000-wide vocab half.  Pure SPMD: all cores run the same
program on different input data.

On-chip layout is "transposed-primary": activations live as hT [E=512, T=1024]
(features on partitions), so every matmul contracts along partitions without
runtime transposes.  LayerNorm statistics are computed with ones-vector matmuls
(partition reduction on the PE) and broadcast back with rank-1 matmuls.
Softmax needs no max-subtraction (scores are provably tiny for this model
family); the causal mask is a 0/1 multiply on diagonal 128x128 tiles, and the
per-query normalizer comes from a ones-column appended to the values in the
attn@v matmul.

Matmul inputs are bf16 (fp32 PSUM accumulation); the residual stream h stays
fp32 in SBUF.  Measured end-to-end error vs the fp32 reference: ~7e-3
scale-relative absmax (numpy emulation).
"""

import contextlib

import numpy as np
import ml_dtypes

import concourse.bass as bass
import concourse.tile as tile
from concourse import bacc, mybir
from concourse.bass_utils import run_bass_kernel_spmd

BF16 = mybir.dt.bfloat16
F32 = mybir.dt.float32
AF = mybir.ActivationFunctionType

B, T, E, H, HD, NB_FULL, EPS, VOCAB = 4, 1024, 512, 8, 64, 5, 1e-5, 32000
NCORES = 8
TC = T // 128    # 8 token chunks
EC = E // 128    # 4 feature chunks


def build_program(NB=NB_FULL, VO=VOCAB // 2, with_kqbias=False, with_projbias=False):
    """Build the SPMD program.  Depends only on shapes/flags (never on values)."""
    nc = bacc.Bacc("TRN2", target_bir_lowering=False, debug=False,
                   enable_asserts=True, num_devices=NCORES)

    NV = (VO + 511) // 512           # number of 512-wide vocab chunks
    last_w = VO - 512 * (NV - 1)     # width of last chunk

    h0T_d = nc.dram_tensor("h0T", [E, T], F32, kind="ExternalInput").ap()
    wk_d = nc.dram_tensor("wk", [NB, E, E], BF16, kind="ExternalInput").ap()
    wq_d = nc.dram_tensor("wq", [NB, E, E], BF16, kind="ExternalInput").ap()
    wres_d = nc.dram_tensor("wres", [NB, E, E], BF16, kind="ExternalInput").ap()
    wmlp_d = nc.dram_tensor("wmlp", [NB, E, E], BF16, kind="ExternalInput").ap()
    mbias_d = nc.dram_tensor("mbias", [NB, E], F32, kind="ExternalInput").ap()
    projw_d = nc.dram_tensor("projw", [E, VO], BF16, kind="ExternalInput").ap()
    mask_d = nc.dram_tensor("mask01", [128, 128], BF16, kind="ExternalInput").ap()
    if with_kqbias:
        kbias_d = nc.dram_tensor("kbias", [NB, E], BF16, kind="ExternalInput").ap()
        qbias_d = nc.dram_tensor("qbias", [NB, E], BF16, kind="ExternalInput").ap()
    if with_projbias:
        projb_d = nc.dram_tensor("projb", [VO], BF16, kind="ExternalInput").ap()
    out_d = nc.dram_tensor("logits", [T, VO], F32, kind="ExternalOutput").ap()

    with tile.TileContext(nc) as tc:
        ctx = contextlib.ExitStack()
        with ctx:
            # SBUF pools (per-partition KB in comments)
            p_h = ctx.enter_context(tc.tile_pool(name="p_h", bufs=1))        # 16
            p_hacc = ctx.enter_context(tc.tile_pool(name="p_hacc", bufs=1))  # 16
            p_hbf = ctx.enter_context(tc.tile_pool(name="p_hbf", bufs=1))    # 8
            p_hn = ctx.enter_context(tc.tile_pool(name="p_hn", bufs=1))      # 8
            p_scr = ctx.enter_context(tc.tile_pool(name="p_scr", bufs=1))    # 14
            p_relu = ctx.enter_context(tc.tile_pool(name="p_relu", bufs=2))  # 8
            p_tmp64 = ctx.enter_context(tc.tile_pool(name="p_tmp64", bufs=2))  # 8
            p_kq = ctx.enter_context(tc.tile_pool(name="p_kq", bufs=1))      # 16
            p_qtok = ctx.enter_context(tc.tile_pool(name="p_qtok", bufs=1))  # ~9
            p_exp = ctx.enter_context(tc.tile_pool(name="p_exp", bufs=2))    # 18
            p_wkq = ctx.enter_context(tc.tile_pool(name="p_wkq", bufs=2))    # 16
            p_wrm = ctx.enter_context(tc.tile_pool(name="p_wrm", bufs=2))    # 16
            p_pw = ctx.enter_context(tc.tile_pool(name="p_pw", bufs=3))      # 12
            p_lout = ctx.enter_context(tc.tile_pool(name="p_lout", bufs=4))  # 8
            p_vec = ctx.enter_context(tc.tile_pool(name="p_vec", bufs=1))    # ~8
            p_rcp = ctx.enter_context(tc.tile_pool(name="p_rcp", bufs=2))    # 4
            p_one = ctx.enter_context(tc.tile_pool(name="p_one", bufs=1))    # ~1
            # PSUM pools: bank-granular shared tags; 4*2KB + 2*4KB = 16KB = 8 banks
            q_b1 = ctx.enter_context(tc.tile_pool(name="q_b1", bufs=4, space="PSUM"))
            q_b2 = ctx.enter_context(tc.tile_pool(name="q_b2", bufs=2, space="PSUM"))

            # ---- static tiles ----
            ones_stat = p_one.tile([128, 1], BF16, tag="ones_stat")
            nc.vector.memset(ones_stat, 1.0)
            eps_sb = p_one.tile([1, 1], F32, tag="eps")
            nc.vector.memset(eps_sb, EPS)
            ones_col = p_one.tile([1, 128], BF16, tag="ones_col")
            nc.vector.memset(ones_col, 1.0)
            ones_col32 = p_one.tile([1, 128], F32, tag="ones_col32")
            nc.vector.memset(ones_col32, 1.0)
            mask_sb = p_one.tile([128, 128], BF16, tag="mask")
            nc.sync.dma_start(out=mask_sb, in_=mask_d[:, :])
            mb_sb = p_one.tile([128, NB * EC], F32, tag="mbias")
            for i in range(NB):
                for m in range(EC):
                    nc.sync.dma_start(
                        out=mb_sb[:, i * EC + m: i * EC + m + 1],
                        in_=mbias_d[i, m * 128:(m + 1) * 128].rearrange("(p o) -> p o", o=1),
                    )
            if with_kqbias:
                kb_sb = p_one.tile([1, NB * E], BF16, tag="kbias")
                qb_sb = p_one.tile([1, NB * E], BF16, tag="qbias")
                nc.sync.dma_start(out=kb_sb, in_=kbias_d[:, :].rearrange("n e -> (n e)").rearrange("(o f) -> o f", o=1))
                nc.sync.dma_start(out=qb_sb, in_=qbias_d[:, :].rearrange("n e -> (n e)").rearrange("(o f) -> o f", o=1))
                ones_row = p_one.tile([1, 512], BF16, tag="ones_row")
                nc.vector.memset(ones_row, 1.0)
            if with_projbias:
                pb_sb = p_one.tile([1, VO], BF16, tag="projb")
                nc.sync.dma_start(out=pb_sb, in_=projb_d[:].rearrange("(o v) -> o v", o=1))

            # ---- initial h ----
            hT = []
            for j in range(EC):
                t_ = p_h.tile([128, T], F32, tag=f"hT{j}")
                nc.sync.dma_start(out=t_, in_=h0T_d[j * 128:(j + 1) * 128, :])
                hT.append(t_)

            def layernorm(hbf_tiles):
                """hnT (bf16 [128,T] x EC) from bf16-cast h tiles."""
                sq = []
                for j in range(EC):
                    s_ = p_scr.tile([128, T], BF16, tag=f"sq{j}")
                    nc.vector.tensor_mul(out=s_, in0=hbf_tiles[j], in1=hbf_tiles[j])
                    sq.append(s_)
                mu_ps = q_b2.tile([1, T], F32, tag="b2")
                s2_ps = q_b2.tile([1, T], F32, tag="b2")
                for th in range(2):
                    sl = slice(th * 512, (th + 1) * 512)
                    for j in range(EC):
                        nc.tensor.matmul(mu_ps[0:1, sl], ones_stat, hbf_tiles[j][:, sl],
                                         start=(j == 0), stop=(j == EC - 1))
                    for j in range(EC):
                        nc.tensor.matmul(s2_ps[0:1, sl], ones_stat, sq[j][:, sl],
                                         start=(j == 0), stop=(j == EC - 1))
                # packed stat vectors: rows 0=mean 1=msq 2=var 3=std 4=rstd
                vecA = p_vec.tile([1, T], F32, tag="vecA")   # mean -> mean*rstd
                vecB = p_vec.tile([1, T], F32, tag="vecB")   # msq -> var -> std -> rstd
                nc.vector.tensor_scalar_mul(out=vecA, in0=mu_ps, scalar1=1.0 / E)
                nc.vector.tensor_mul(out=vecB, in0=vecA, in1=vecA)
                nc.vector.tensor_scalar_mul(out=s2_ps, in0=s2_ps, scalar1=1.0 / E)
                nc.vector.tensor_sub(out=vecB, in0=s2_ps, in1=vecB)
                nc.scalar.activation(out=vecB, in_=vecB, func=AF.Sqrt, bias=eps_sb)
                nc.vector.reciprocal(out=vecB, in_=vecB)
                nc.vector.tensor_mul(out=vecA, in0=vecA, in1=vecB)
                # broadcast via f32 rank-1 matmuls (keeps per-token scale exact)
                a_bc = q_b2.tile([128, T], F32, tag="b2")
                b_bc = q_b2.tile([128, T], F32, tag="b2")
                for th in range(2):
                    sl = slice(th * 512, (th + 1) * 512)
                    nc.tensor.matmul(a_bc[:, sl], ones_col32, vecB[0:1, sl],
                                     start=True, stop=True)
                    nc.tensor.matmul(b_bc[:, sl], ones_col32, vecA[0:1, sl],
                                     start=True, stop=True)
                a_sb = p_scr.tile([128, T], F32, tag="a_sb")
                b_sb = p_scr.tile([128, T], F32, tag="b_sb")
                nc.vector.tensor_copy(out=a_sb, in_=a_bc)
                nc.vector.tensor_copy(out=b_sb, in_=b_bc)
                hn = []
                for j in range(EC):
                    t1 = p_scr.tile([128, T], F32, tag="t1")
                    nc.vector.tensor_mul(out=t1, in0=hbf_tiles[j], in1=a_sb)
                    hn_j = p_hn.tile([128, T], BF16, tag=f"hn{j}")
                    # hn = h*rstd - mean*rstd (single bf16 rounding at the end)
                    nc.vector.tensor_sub(out=hn_j, in0=t1, in1=b_sb)
                    hn.append(hn_j)
                return hn

            def cast_bf(tiles):
                out = []
                for j in range(EC):
                    c_ = p_hbf.tile([128, T], BF16, tag=f"hbf{j}")
                    nc.gpsimd.tensor_copy(out=c_, in_=tiles[j])
                    out.append(c_)
                return out

            for i in range(NB):
                # ---- stream weights for this block ----
                wres_sb, wk_sb, wq_sb, wmlp_sb = [], [], [], []
                for j in range(EC):
                    w_ = p_wrm.tile([128, E], BF16, tag=f"wres{j}")
                    nc.sync.dma_start(out=w_, in_=wres_d[i, j * 128:(j + 1) * 128, :])
                    wres_sb.append(w_)
                for j in range(EC):
                    w_ = p_wkq.tile([128, E], BF16, tag=f"wk{j}")
                    nc.sync.dma_start(out=w_, in_=wk_d[i, j * 128:(j + 1) * 128, :])
                    wk_sb.append(w_)
                for j in range(EC):
                    w_ = p_wkq.tile([128, E], BF16, tag=f"wq{j}")
                    nc.sync.dma_start(out=w_, in_=wq_d[i, j * 128:(j + 1) * 128, :])
                    wq_sb.append(w_)
                for j in range(EC):
                    w_ = p_wrm.tile([128, E], BF16, tag=f"wmlp{j}")
                    nc.sync.dma_start(out=w_, in_=wmlp_d[i, j * 128:(j + 1) * 128, :])
                    wmlp_sb.append(w_)

                hbf = cast_bf(hT)

                # ---- Wres: wacc_m = (h @ Wres)^T chunk m ----
                wacc = []
                for m in range(EC):
                    pw = q_b2.tile([128, T], F32, tag="b2")
                    for th in range(2):
                        sl = slice(th * 512, (th + 1) * 512)
                        for j in range(EC):
                            nc.tensor.matmul(pw[:, sl],
                                             wres_sb[j][:, m * 128:(m + 1) * 128],
                                             hbf[j][:, sl],
                                             start=(j == 0), stop=(j == EC - 1))
                    wa = p_hacc.tile([128, T], F32, tag=f"hacc{m}")
                    nc.vector.tensor_copy(out=wa, in_=pw)
                    wacc.append(wa)

                # ---- LN1 -> hnT ----
                hn = layernorm(hbf)

                # ---- kT, qT ([d,t] layout, 2 heads per 128-partition tile) ----
                kT, qT = [], []
                for m in range(EC):
                    kT.append(p_kq.tile([128, T], BF16, tag=f"kT{m}", name=f"kT{m}"))
                    qT.append(p_kq.tile([128, T], BF16, tag=f"qT{m}", name=f"qT{m}"))
                for m in range(EC):
                    for th in range(2):
                        sl = slice(th * 512, (th + 1) * 512)
                        pk = q_b1.tile([128, 512], F32, tag="b1")
                        for j in range(EC):
                            nc.tensor.matmul(pk,
                                             wk_sb[j][:, m * 128:(m + 1) * 128],
                                             hn[j][:, sl],
                                             start=(j == 0),
                                             stop=(j == EC - 1) and not with_kqbias)
                        if with_kqbias:
                            nc.tensor.matmul(pk,
                                             kb_sb[0:1, i * E + m * 128: i * E + (m + 1) * 128],
                                             ones_row,
                                             start=False, stop=True)
                        nc.scalar.activation(out=kT[m][:, sl], in_=pk, func=AF.Copy)
                        pq = q_b1.tile([128, 512], F32, tag="b1")
                        for j in range(EC):
                            nc.tensor.matmul(pq,
                                             wq_sb[j][:, m * 128:(m + 1) * 128],
                                             hn[j][:, sl],
                                             start=(j == 0),
                                             stop=(j == EC - 1) and not with_kqbias)
                        if with_kqbias:
                            nc.tensor.matmul(pq,
                                             qb_sb[0:1, i * E + m * 128: i * E + (m + 1) * 128],
                                             ones_row,
                                             start=False, stop=True)
                        nc.vector.tensor_copy(out=qT[m][:, sl], in_=pq)

                # ---- q_tok ([s, (h,d)] layout + ones column per head) ----
                qtok = []
                for t in range(TC):
                    qt = p_qtok.tile([128, H, HD + 1], BF16, tag=f"qtok{t}")
                    pq2 = q_b1.tile([128, 512], F32, tag="b1")
                    for j in range(EC):
                        nc.tensor.matmul(pq2,
                                         hn[j][:, t * 128:(t + 1) * 128],
                                         wq_sb[j],
                                         start=(j == 0),
                                         stop=(j == EC - 1) and not with_kqbias)
                    if with_kqbias:
                        nc.tensor.matmul(pq2, ones_col,
                                         qb_sb[0:1, i * E:(i + 1) * E],
                                         start=False, stop=True)
                    nc.vector.tensor_copy(
                        out=qt[:, :, 0:HD],
                        in_=pq2.rearrange("p (h d) -> p h d", h=H))
                    nc.vector.memset(qt[:, :, HD:HD + 1], 1.0)
                    qtok.append(qt)

                # ---- attention, head by head ----
                for h in range(H):
                    m, off = h // 2, (h % 2) * HD
                    etiles = []
                    for sc in range(TC):
                        base = sc * 128
                        et = p_exp.tile([128, T - base], BF16, tag=f"e{sc}")
                        etiles.append((et, base))
                        th0 = base // 512
                        for th in range(th0, 2):
                            t0 = max(th * 512, base)
                            t1 = (th + 1) * 512
                            ps_s = q_b1.tile([128, 512], F32, tag="b1")
                            nc.tensor.matmul(
                                ps_s[:, 0:t1 - t0],
                                qT[m][off:off + HD, base:base + 128],
                                kT[m][off:off + HD, t0:t1],
                                start=True, stop=True)
                            nc.scalar.activation(out=et[:, t0 - base:t1 - base],
                                                 in_=ps_s[:, 0:t1 - t0], func=AF.Exp)
                        # causal mask on the diagonal tile
                        nc.gpsimd.tensor_mul(out=et[:, 0:128], in0=et[:, 0:128],
                                             in1=mask_sb)

                    pov = q_b2.tile([HD + 1, T], F32, tag="b2")
                    for th in range(2):
                        scs = [sc for sc in range(TC) if sc * 128 < (th + 1) * 512]
                        for si, sc in enumerate(scs):
                            t0 = max(th * 512, sc * 128)
                            t1 = (th + 1) * 512
                            et, base = etiles[sc]
                            nc.tensor.matmul(
                                pov[:, t0:t1],
                                qtok[sc][:, h, :],
                                et[:, t0 - base:t1 - base],
                                start=(si == 0), stop=(si == len(scs) - 1))
                    rcp = p_rcp.tile([1, T], BF16, tag="rcp")
                    with nc.allow_low_precision(reason="softmax normalizer in bf16"):
                        nc.vector.reciprocal(out=rcp, in_=pov[HD:HD + 1, :])
                    rbc = q_b2.tile([128, T], F32, tag="b2")
                    for th in range(2):
                        sl = slice(th * 512, (th + 1) * 512)
                        nc.tensor.matmul(rbc[:, sl], ones_col,
                                         rcp[0:1, sl], start=True, stop=True)
                    tmp = p_tmp64.tile([128, T], F32, tag="tmp")
                    nc.vector.tensor_copy(out=tmp[off:off + HD, :], in_=pov[0:HD, :])
                    nc.vector.tensor_mul(out=tmp[off:off + HD, :],
                                         in0=tmp[off:off + HD, :],
                                         in1=rbc[off:off + HD, :])
                    nc.gpsimd.tensor_add(out=wacc[m][off:off + HD, :],
                                         in0=wacc[m][off:off + HD, :],
                                         in1=tmp[off:off + HD, :])

                # ---- LN2 on h_new (= wacc) ----
                hbf2 = cast_bf(wacc)
                hn2 = layernorm(hbf2)

                # ---- MLP ----
                hT_next = []
                for m in range(EC):
                    pm = q_b2.tile([128, T], F32, tag="b2")
                    for th in range(2):
                        sl = slice(th * 512, (th + 1) * 512)
                        for j in range(EC):
                            nc.tensor.matmul(pm[:, sl],
                                             wmlp_sb[j][:, m * 128:(m + 1) * 128],
                                             hn2[j][:, sl],
                                             start=(j == 0), stop=(j == EC - 1))
                    relu_t = p_relu.tile([128, T], F32, tag="relu")
                    nc.scalar.activation(out=relu_t, in_=pm, func=AF.Relu,
                                         bias=mb_sb[:, i * EC + m: i * EC + m + 1])
                    hnew = p_h.tile([128, T], F32, tag=f"hT{m}")
                    nc.gpsimd.tensor_add(out=hnew, in0=wacc[m], in1=relu_t)
                    hT_next.append(hnew)
                hT = hT_next

            # ---- final LN + projection ----
            hbf_f = cast_bf(hT)
            hnf = layernorm(hbf_f)

            for vc in range(NV):
                vw = 512 if vc < NV - 1 else last_w
                pw_sb = []
                for j in range(EC):
                    w_ = p_pw.tile([128, 512], BF16, tag=f"pw{j}")
                    nc.sync.dma_start(
                        out=w_[:, 0:vw],
                        in_=projw_d[j * 128:(j + 1) * 128, vc * 512:vc * 512 + vw])
                    pw_sb.append(w_)
                for t in range(TC):
                    pl = q_b1.tile([128, 512], F32, tag="b1")
                    for j in range(EC):
                        nc.tensor.matmul(pl[:, 0:vw],
                                         hnf[j][:, t * 128:(t + 1) * 128],
                                         pw_sb[j][:, 0:vw],
                                         start=(j == 0),
                                         stop=(j == EC - 1) and not with_projbias)
                    if with_projbias:
                        nc.tensor.matmul(pl[:, 0:vw], ones_col,
                                         pb_sb[0:1, vc * 512:vc * 512 + vw],
                                         start=False, stop=True)
                    lo = p_lout.tile([128, 512], F32, tag="lo")
                    if (vc + t) % 2 == 0:
                        nc.vector.tensor_copy(out=lo[:, 0:vw], in_=pl[:, 0:vw])
                    else:
                        nc.scalar.activation(out=lo[:, 0:vw], in_=pl[:, 0:vw],
                                             func=AF.Copy)
                    nc.sync.dma_start(
                        out=out_d[t * 128:(t + 1) * 128, vc * 512:vc * 512 + vw],
                        in_=lo[:, 0:vw])

    nc.compile()
    return nc


def prepare_inputs(inputs, NB=NB_FULL, VO=VOCAB // 2):
    """Host-side prep: fold LN gains/scale into weights, build per-core maps."""
    f = lambda a: np.asarray(a, dtype=np.float32)
    x = np.asarray(inputs["x"]).astype(np.int64)
    tok_emb, pos_emb = f(inputs["tok_emb"]), f(inputs["pos_emb"])
    Wk, Wq, Wres = f(inputs["Wk"]), f(inputs["Wq"]), f(inputs["Wres"])
    ln1_g, ln1_b = f(inputs["ln1_g"]), f(inputs["ln1_b"])
    mlp_W, mlp_b = f(inputs["mlp_W"]), f(inputs["mlp_b"])
    ln2_g, ln2_b = f(inputs["ln2_g"]), f(inputs["ln2_b"])
    lnf_g, lnf_b = f(inputs["lnf_g"]), f(inputs["lnf_b"])
    proj_W, proj_b = f(inputs["proj_W"]), f(inputs["proj_b"])

    bf = lambda a: np.ascontiguousarray(a).astype(ml_dtypes.bfloat16)
    scale = np.float32(E) ** -0.5

    # [H,E,HD] -> [E, H*HD], fold ln gain (rows) and score scale into Wk
    wk_eff = np.stack([(Wk[i].transpose(1, 0, 2).reshape(E, E)
                        * ln1_g[i][:, None] * scale) for i in range(NB)])
    wq_eff = np.stack([(Wq[i].transpose(1, 0, 2).reshape(E, E)
                        * ln1_g[i][:, None]) for i in range(NB)])
    wres_eff = Wres[:NB]
    wmlp_eff = np.stack([mlp_W[i] * ln2_g[i][:, None] for i in range(NB)])
    mbias_eff = np.stack([ln2_b[i] @ mlp_W[i] + mlp_b[i] for i in range(NB)])
    projw_eff = proj_W * lnf_g[:, None]
    projb_eff = proj_b + lnf_b @ proj_W

    kbias = np.stack([ln1_b[i] @ (Wk[i].transpose(1, 0, 2).reshape(E, E) * scale)
                      for i in range(NB)])
    qbias = np.stack([ln1_b[i] @ Wq[i].transpose(1, 0, 2).reshape(E, E)
                      for i in range(NB)])
    with_kqbias = bool(np.any(kbias) or np.any(qbias))
    with_projbias = bool(np.any(projb_eff))

    mask01 = np.ascontiguousarray(np.tril(np.ones((128, 128), np.float32)).T)

    common = {
        "wk": bf(wk_eff), "wq": bf(wq_eff), "wres": bf(wres_eff),
        "wmlp": bf(wmlp_eff), "mbias": np.ascontiguousarray(mbias_eff),
        "mask01": bf(mask01),
    }
    if with_kqbias:
        common["kbias"] = bf(kbias)
        common["qbias"] = bf(qbias)

    in_maps = []
    for c in range(NCORES):
        b, jv = c // 2, c % 2
        emb = tok_emb[x[b]] + pos_emb[:T]
        m = dict(common)
        m["h0T"] = np.ascontiguousarray(emb.T.astype(np.float32))
        m["projw"] = bf(projw_eff[:, jv * VO:(jv + 1) * VO])
        if with_projbias:
            m["projb"] = bf(projb_eff[jv * VO:(jv + 1) * VO])
        in_maps.append(m)
    return in_maps, with_kqbias, with_projbias


_PROGRAM_CACHE = {}


def kernel(**inputs):
    VO = VOCAB // 2
    in_maps, with_kqbias, with_projbias = prepare_inputs(inputs, NB_FULL, VO)
    key = (NB_FULL, VO, with_kqbias, with_projbias)
    if key not in _PROGRAM_CACHE:
        _PROGRAM_CACHE[key] = build_program(NB_FULL, VO, with_kqbias, with_projbias)
    nc = _PROGRAM_CACHE[key]
    res = run_bass_kernel_spmd(nc, in_maps, list(range(NCORES))).results
    out = np.empty((B, T, VOCAB), np.float32)
    for c in range(NCORES):
        b, jv = c // 2, c % 2
        out[b, :, jv * VO:(jv + 1) * VO] = res[c]["logits"]
    return out


# revision 23
# speedup vs baseline: 2.1654x; 2.1654x over previous
"""Trainium2 Bass kernel for nn_PoetryModel (dense transformer, B=4 T=1024 E=512
H=8 HD=64 NB=5 VOCAB=32000), 8 NeuronCores.

Sharding: core c = 2*b + jv handles batch b = c//2 and vocab half jv = c%2.
Each core runs the full 5-block transformer for its sequence (block compute is
duplicated across the pair -- no cross-core communication needed), then projects
its sequence onto its 16000-wide vocab half.  Pure SPMD: all cores run the same
program on different input data.

On-chip layout is "transposed-primary": activations live as hT [E=512, T=1024]
(features on partitions), so every matmul contracts along partitions without
runtime transposes.  LayerNorm statistics are computed with ones-vector matmuls
(partition reduction on the PE) and broadcast back with rank-1 matmuls.
Softmax needs no max-subtraction (scores are provably tiny for this model
family); the causal mask is a 0/1 multiply on diagonal 128x128 tiles, and the
per-query normalizer comes from a ones-column appended to the values in the
attn@v matmul.

Matmul inputs are bf16 (fp32 PSUM accumulation); the residual stream h stays
fp32 in SBUF.  Measured end-to-end error vs the fp32 reference: ~7e-3
scale-relative absmax (numpy emulation).
"""

import contextlib

import numpy as np
import ml_dtypes

import concourse.bass as bass
import concourse.tile as tile
from concourse import bacc, mybir
from concourse.bass_utils import run_bass_kernel_spmd

BF16 = mybir.dt.bfloat16
F32 = mybir.dt.float32
AF = mybir.ActivationFunctionType

B, T, E, H, HD, NB_FULL, EPS, VOCAB = 4, 1024, 512, 8, 64, 5, 1e-5, 32000
NCORES = 8
TC = T // 128    # 8 token chunks
EC = E // 128    # 4 feature chunks


def build_program(NB=NB_FULL, VO=VOCAB // 2, with_kqbias=False, with_projbias=False):
    """Build the SPMD program.  Depends only on shapes/flags (never on values)."""
    nc = bacc.Bacc("TRN2", target_bir_lowering=False, debug=False,
                   enable_asserts=True, num_devices=NCORES)

    NV = (VO + 511) // 512           # number of 512-wide vocab chunks
    last_w = VO - 512 * (NV - 1)     # width of last chunk

    h0T_d = nc.dram_tensor("h0T", [E, T], F32, kind="ExternalInput").ap()
    wk_d = nc.dram_tensor("wk", [NB, E, E], BF16, kind="ExternalInput").ap()
    wq_d = nc.dram_tensor("wq", [NB, E, E], BF16, kind="ExternalInput").ap()
    wres_d = nc.dram_tensor("wres", [NB, E, E], BF16, kind="ExternalInput").ap()
    wmlp_d = nc.dram_tensor("wmlp", [NB, E, E], BF16, kind="ExternalInput").ap()
    mbias_d = nc.dram_tensor("mbias", [NB, E], F32, kind="ExternalInput").ap()
    projw_d = nc.dram_tensor("projw", [E, VO], BF16, kind="ExternalInput").ap()
    mask_d = nc.dram_tensor("mask01", [128, 128], BF16, kind="ExternalInput").ap()
    if with_kqbias:
        kbias_d = nc.dram_tensor("kbias", [NB, E], BF16, kind="ExternalInput").ap()
        qbias_d = nc.dram_tensor("qbias", [NB, E], BF16, kind="ExternalInput").ap()
    if with_projbias:
        projb_d = nc.dram_tensor("projb", [VO], BF16, kind="ExternalInput").ap()
    out_d = nc.dram_tensor("logits", [T, VO], F32, kind="ExternalOutput").ap()

    with tile.TileContext(nc) as tc:
        ctx = contextlib.ExitStack()
        with ctx:
            # SBUF pools (per-partition KB in comments)
            p_h = ctx.enter_context(tc.tile_pool(name="p_h", bufs=1))        # 16
            p_hacc = ctx.enter_context(tc.tile_pool(name="p_hacc", bufs=1))  # 16
            p_hbf = ctx.enter_context(tc.tile_pool(name="p_hbf", bufs=1))    # 8
            p_hn = ctx.enter_context(tc.tile_pool(name="p_hn", bufs=1))      # 8
            p_scr = ctx.enter_context(tc.tile_pool(name="p_scr", bufs=1))    # 14
            p_relu = ctx.enter_context(tc.tile_pool(name="p_relu", bufs=2))  # 8
            p_tmp64 = ctx.enter_context(tc.tile_pool(name="p_tmp64", bufs=2))  # 8
            p_kq = ctx.enter_context(tc.tile_pool(name="p_kq", bufs=1))      # 16
            p_qtok = ctx.enter_context(tc.tile_pool(name="p_qtok", bufs=1))  # ~9
            p_exp = ctx.enter_context(tc.tile_pool(name="p_exp", bufs=2))    # 18
            p_wkq = ctx.enter_context(tc.tile_pool(name="p_wkq", bufs=2))    # 16
            p_wrm = ctx.enter_context(tc.tile_pool(name="p_wrm", bufs=2))    # 16
            p_pw = ctx.enter_context(tc.tile_pool(name="p_pw", bufs=3))      # 12
            p_lout = ctx.enter_context(tc.tile_pool(name="p_lout", bufs=4))  # 8
            p_vec = ctx.enter_context(tc.tile_pool(name="p_vec", bufs=1))    # ~8
            p_rcp = ctx.enter_context(tc.tile_pool(name="p_rcp", bufs=2))    # 4
            p_one = ctx.enter_context(tc.tile_pool(name="p_one", bufs=1))    # ~1
            # PSUM pools: bank-granular shared tags; 4*2KB + 2*4KB = 16KB = 8 banks
            q_b1 = ctx.enter_context(tc.tile_pool(name="q_b1", bufs=4, space="PSUM"))
            q_b2 = ctx.enter_context(tc.tile_pool(name="q_b2", bufs=2, space="PSUM"))

            # ---- static tiles ----
            ones_stat = p_one.tile([128, 1], BF16, tag="ones_stat")
            nc.vector.memset(ones_stat, 1.0)
            eps_sb = p_one.tile([1, 1], F32, tag="eps")
            nc.vector.memset(eps_sb, EPS)
            ones_col = p_one.tile([1, 128], BF16, tag="ones_col")
            nc.vector.memset(ones_col, 1.0)
            ones_col32 = p_one.tile([1, 128], F32, tag="ones_col32")
            nc.vector.memset(ones_col32, 1.0)
            mask_sb = p_one.tile([128, 128], BF16, tag="mask")
            nc.sync.dma_start(out=mask_sb, in_=mask_d[:, :])
            mb_sb = p_one.tile([128, NB * EC], F32, tag="mbias")
            for i in range(NB):
                for m in range(EC):
                    nc.sync.dma_start(
                        out=mb_sb[:, i * EC + m: i * EC + m + 1],
                        in_=mbias_d[i, m * 128:(m + 1) * 128].rearrange("(p o) -> p o", o=1),
                    )
            if with_kqbias:
                kb_sb = p_one.tile([1, NB * E], BF16, tag="kbias")
                qb_sb = p_one.tile([1, NB * E], BF16, tag="qbias")
                nc.sync.dma_start(out=kb_sb, in_=kbias_d[:, :].rearrange("n e -> (n e)").rearrange("(o f) -> o f", o=1))
                nc.sync.dma_start(out=qb_sb, in_=qbias_d[:, :].rearrange("n e -> (n e)").rearrange("(o f) -> o f", o=1))
                ones_row = p_one.tile([1, 512], BF16, tag="ones_row")
                nc.vector.memset(ones_row, 1.0)
            if with_projbias:
                pb_sb = p_one.tile([1, VO], BF16, tag="projb")
                nc.sync.dma_start(out=pb_sb, in_=projb_d[:].rearrange("(o v) -> o v", o=1))

            # ---- initial h ----
            hT = []
            for j in range(EC):
                t_ = p_h.tile([128, T], F32, tag=f"hT{j}")
                nc.sync.dma_start(out=t_, in_=h0T_d[j * 128:(j + 1) * 128, :])
                hT.append(t_)

            def layernorm(hbf_tiles):
                """hnT (bf16 [128,T] x EC) from bf16-cast h tiles."""
                sq = []
                for j in range(EC):
                    s_ = p_scr.tile([128, T], BF16, tag=f"sq{j}")
                    nc.vector.tensor_mul(out=s_, in0=hbf_tiles[j], in1=hbf_tiles[j])
                    sq.append(s_)
                mu_ps = q_b2.tile([1, T], F32, tag="b2")
                s2_ps = q_b2.tile([1, T], F32, tag="b2")
                for th in range(2):
                    sl = slice(th * 512, (th + 1) * 512)
                    for j in range(EC):
                        nc.tensor.matmul(mu_ps[0:1, sl], ones_stat, hbf_tiles[j][:, sl],
                                         start=(j == 0), stop=(j == EC - 1))
                    for j in range(EC):
                        nc.tensor.matmul(s2_ps[0:1, sl], ones_stat, sq[j][:, sl],
                                         start=(j == 0), stop=(j == EC - 1))
                # packed stat vectors: rows 0=mean 1=msq 2=var 3=std 4=rstd
                vecA = p_vec.tile([1, T], F32, tag="vecA")   # mean -> mean*rstd
                vecB = p_vec.tile([1, T], F32, tag="vecB")   # msq -> var -> std -> rstd
                nc.vector.tensor_scalar_mul(out=vecA, in0=mu_ps, scalar1=1.0 / E)
                nc.vector.tensor_mul(out=vecB, in0=vecA, in1=vecA)
                nc.vector.tensor_scalar_mul(out=s2_ps, in0=s2_ps, scalar1=1.0 / E)
                nc.vector.tensor_sub(out=vecB, in0=s2_ps, in1=vecB)
                nc.scalar.activation(out=vecB, in_=vecB, func=AF.Sqrt, bias=eps_sb)
                nc.vector.reciprocal(out=vecB, in_=vecB)
                nc.vector.tensor_mul(out=vecA, in0=vecA, in1=vecB)
                # broadcast via f32 rank-1 matmuls (keeps per-token scale exact)
                a_bc = q_b2.tile([128, T], F32, tag="b2")
                b_bc = q_b2.tile([128, T], F32, tag="b2")
                for th in range(2):
                    sl = slice(th * 512, (th + 1) * 512)
                    nc.tensor.matmul(a_bc[:, sl], ones_col32, vecB[0:1, sl],
                                     start=True, stop=True)
                    nc.tensor.matmul(b_bc[:, sl], ones_col32, vecA[0:1, sl],
                                     start=True, stop=True)
                a_sb = p_scr.tile([128, T], F32, tag="a_sb")
                b_sb = p_scr.tile([128, T], F32, tag="b_sb")
                nc.vector.tensor_copy(out=a_sb, in_=a_bc)
                nc.vector.tensor_copy(out=b_sb, in_=b_bc)
                hn = []
                for j in range(EC):
                    t1 = p_scr.tile([128, T], F32, tag="t1")
                    nc.vector.tensor_mul(out=t1, in0=hbf_tiles[j], in1=a_sb)
                    hn_j = p_hn.tile([128, T], BF16, tag=f"hn{j}")
                    # hn = h*rstd - mean*rstd (single bf16 rounding at the end)
                    nc.vector.tensor_sub(out=hn_j, in0=t1, in1=b_sb)
                    hn.append(hn_j)
                return hn

            def cast_bf(tiles):
                out = []
                for j in range(EC):
                    c_ = p_hbf.tile([128, T], BF16, tag=f"hbf{j}")
                    nc.gpsimd.tensor_copy(out=c_, in_=tiles[j])
                    out.append(c_)
                return out

            for i in range(NB):
                # ---- stream weights for this block ----
                wres_sb, wk_sb, wq_sb, wmlp_sb = [], [], [], []
                for j in range(EC):
                    w_ = p_wrm.tile([128, E], BF16, tag=f"wres{j}")
                    nc.sync.dma_start(out=w_, in_=wres_d[i, j * 128:(j + 1) * 128, :])
                    wres_sb.append(w_)
                for j in range(EC):
                    w_ = p_wkq.tile([128, E], BF16, tag=f"wk{j}")
                    nc.sync.dma_start(out=w_, in_=wk_d[i, j * 128:(j + 1) * 128, :])
                    wk_sb.append(w_)
                for j in range(EC):
                    w_ = p_wkq.tile([128, E], BF16, tag=f"wq{j}")
                    nc.sync.dma_start(out=w_, in_=wq_d[i, j * 128:(j + 1) * 128, :])
                    wq_sb.append(w_)
                for j in range(EC):
                    w_ = p_wrm.tile([128, E], BF16, tag=f"wmlp{j}")
                    nc.sync.dma_start(out=w_, in_=wmlp_d[i, j * 128:(j + 1) * 128, :])
                    wmlp_sb.append(w_)

                hbf = cast_bf(hT)

                # ---- Wres: wacc_m = (h @ Wres)^T chunk m ----
                wacc = []
                for m in range(EC):
                    pw = q_b2.tile([128, T], F32, tag="b2")
                    for th in range(2):
                        sl = slice(th * 512, (th + 1) * 512)
                        for j in range(EC):
                            nc.tensor.matmul(pw[:, sl],
                                             wres_sb[j][:, m * 128:(m + 1) * 128],
                                             hbf[j][:, sl],
                                             start=(j == 0), stop=(j == EC - 1))
                    wa = p_hacc.tile([128, T], F32, tag=f"hacc{m}")
                    nc.vector.tensor_copy(out=wa, in_=pw)
                    wacc.append(wa)

                # ---- LN1 -> hnT ----
                hn = layernorm(hbf)

                # ---- kT, qT ([d,t] layout, 2 heads per 128-partition tile) ----
                kT, qT = [], []
                for m in range(EC):
                    kT.append(p_kq.tile([128, T], BF16, tag=f"kT{m}", name=f"kT{m}"))
                    qT.append(p_kq.tile([128, T], BF16, tag=f"qT{m}", name=f"qT{m}"))
                for m in range(EC):
                    for th in range(2):
                        sl = slice(th * 512, (th + 1) * 512)
                        pk = q_b1.tile([128, 512], F32, tag="b1")
                        for j in range(EC):
                            nc.tensor.matmul(pk,
                                             wk_sb[j][:, m * 128:(m + 1) * 128],
                                             hn[j][:, sl],
                                             start=(j == 0),
                                             stop=(j == EC - 1) and not with_kqbias)
                        if with_kqbias:
                            nc.tensor.matmul(pk,
                                             kb_sb[0:1, i * E + m * 128: i * E + (m + 1) * 128],
                                             ones_row,
                                             start=False, stop=True)
                        nc.scalar.activation(out=kT[m][:, sl], in_=pk, func=AF.Copy)
                        pq = q_b1.tile([128, 512], F32, tag="b1")
                        for j in range(EC):
                            nc.tensor.matmul(pq,
                                             wq_sb[j][:, m * 128:(m + 1) * 128],
                                             hn[j][:, sl],
                                             start=(j == 0),
                                             stop=(j == EC - 1) and not with_kqbias)
                        if with_kqbias:
                            nc.tensor.matmul(pq,
                                             qb_sb[0:1, i * E + m * 128: i * E + (m + 1) * 128],
                                             ones_row,
                                             start=False, stop=True)
                        nc.vector.tensor_copy(out=qT[m][:, sl], in_=pq)

                # ---- q_tok ([s, (h,d)] layout + ones column per head) ----
                qtok = []
                for t in range(TC):
                    qt = p_qtok.tile([128, H, HD + 1], BF16, tag=f"qtok{t}")
                    pq2 = q_b1.tile([128, 512], F32, tag="b1")
                    for j in range(EC):
                        nc.tensor.matmul(pq2,
                                         hn[j][:, t * 128:(t + 1) * 128],
                                         wq_sb[j],
                                         start=(j == 0),
                                         stop=(j == EC - 1) and not with_kqbias)
                    if with_kqbias:
                        nc.tensor.matmul(pq2, ones_col,
                                         qb_sb[0:1, i * E:(i + 1) * E],
                                         start=False, stop=True)
                    nc.vector.tensor_copy(
                        out=qt[:, :, 0:HD],
                        in_=pq2.rearrange("p (h d) -> p h d", h=H))
                    nc.vector.memset(qt[:, :, HD:HD + 1], 1.0)
                    qtok.append(qt)

                # ---- attention, head by head ----
                for h in range(H):
                    m, off = h // 2, (h % 2) * HD
                    etiles = []
                    for sc in range(TC):
                        base = sc * 128
                        et = p_exp.tile([128, T - base], BF16, tag=f"e{sc}")
                        etiles.append((et, base))
                        th0 = base // 512
                        for th in range(th0, 2):
                            t0 = max(th * 512, base)
                            t1 = (th + 1) * 512
                            ps_s = q_b1.tile([128, 512], F32, tag="b1")
                            nc.tensor.matmul(
                                ps_s[:, 0:t1 - t0],
                                qT[m][off:off + HD, base:base + 128],
                                kT[m][off:off + HD, t0:t1],
                                start=True, stop=True)
                            nc.scalar.activation(out=et[:, t0 - base:t1 - base],
                                                 in_=ps_s[:, 0:t1 - t0], func=AF.Exp)
                        # causal mask on the diagonal tile
                        nc.gpsimd.tensor_mul(out=et[:, 0:128], in0=et[:, 0:128],
                                             in1=mask_sb)

                    pov = q_b2.tile([HD + 1, T], F32, tag="b2")
                    for th in range(2):
                        scs = [sc for sc in range(TC) if sc * 128 < (th + 1) * 512]
                        for si, sc in enumerate(scs):
                            t0 = max(th * 512, sc * 128)
                            t1 = (th + 1) * 512
                            et, base = etiles[sc]
                            nc.tensor.matmul(
                                pov[:, t0:t1],
                                qtok[sc][:, h, :],
                                et[:, t0 - base:t1 - base],
                                start=(si == 0), stop=(si == len(scs) - 1))
                    rcp = p_rcp.tile([1, T], BF16, tag="rcp")
                    with nc.allow_low_precision(reason="softmax normalizer in bf16"):
                        nc.vector.reciprocal(out=rcp, in_=pov[HD:HD + 1, :])
                    rbc = q_b2.tile([128, T], F32, tag="b2")
                    for th in range(2):
                        sl = slice(th * 512, (th + 1) * 512)
                        nc.tensor.matmul(rbc[:, sl], ones_col,
                                         rcp[0:1, sl], start=True, stop=True)
                    tmp = p_tmp64.tile([128, T], F32, tag="tmp")
                    nc.vector.tensor_copy(out=tmp[off:off + HD, :], in_=pov[0:HD, :])
                    nc.vector.tensor_mul(out=tmp[off:off + HD, :],
                                         in0=tmp[off:off + HD, :],
                                         in1=rbc[off:off + HD, :])
                    nc.gpsimd.tensor_add(out=wacc[m][off:off + HD, :],
                                         in0=wacc[m][off:off + HD, :],
                                         in1=tmp[off:off + HD, :])

                # ---- LN2 on h_new (= wacc) ----
                hbf2 = cast_bf(wacc)
                hn2 = layernorm(hbf2)

                # ---- MLP ----
                hT_next = []
                for m in range(EC):
                    pm = q_b2.tile([128, T], F32, tag="b2")
                    for th in range(2):
                        sl = slice(th * 512, (th + 1) * 512)
                        for j in range(EC):
                            nc.tensor.matmul(pm[:, sl],
                                             wmlp_sb[j][:, m * 128:(m + 1) * 128],
                                             hn2[j][:, sl],
                                             start=(j == 0), stop=(j == EC - 1))
                    relu_t = p_relu.tile([128, T], F32, tag="relu")
                    nc.scalar.activation(out=relu_t, in_=pm, func=AF.Relu,
                                         bias=mb_sb[:, i * EC + m: i * EC + m + 1])
                    hnew = p_h.tile([128, T], F32, tag=f"hT{m}")
                    nc.gpsimd.tensor_add(out=hnew, in0=wacc[m], in1=relu_t)
                    hT_next.append(hnew)
                hT = hT_next

            # ---- final LN + projection ----
            hbf_f = cast_bf(hT)
            hnf = layernorm(hbf_f)

            for vc in range(NV):
                vw = 512 if vc < NV - 1 else last_w
                pw_sb = []
                for j in range(EC):
                    w_ = p_pw.tile([128, 512], BF16, tag=f"pw{j}")
                    nc.sync.dma_start(
                        out=w_[:, 0:vw],
                        in_=projw_d[j * 128:(j + 1) * 128, vc * 512:vc * 512 + vw])
                    pw_sb.append(w_)
                for t in range(TC):
                    pl = q_b1.tile([128, 512], F32, tag="b1")
                    for j in range(EC):
                        nc.tensor.matmul(pl[:, 0:vw],
                                         hnf[j][:, t * 128:(t + 1) * 128],
                                         pw_sb[j][:, 0:vw],
                                         start=(j == 0),
                                         stop=(j == EC - 1) and not with_projbias)
                    if with_projbias:
                        nc.tensor.matmul(pl[:, 0:vw], ones_col,
                                         pb_sb[0:1, vc * 512:vc * 512 + vw],
                                         start=False, stop=True)
                    lo = p_lout.tile([128, 512], F32, tag="lo")
                    if (vc + t) % 2 == 0:
                        nc.vector.tensor_copy(out=lo[:, 0:vw], in_=pl[:, 0:vw])
                    else:
                        nc.scalar.activation(out=lo[:, 0:vw], in_=pl[:, 0:vw],
                                             func=AF.Copy)
                    nc.sync.dma_start(
                        out=out_d[t * 128:(t + 1) * 128, vc * 512:vc * 512 + vw],
                        in_=lo[:, 0:vw])

    nc.compile()
    return nc


def prepare_inputs(inputs, NB=NB_FULL, VO=VOCAB // 2):
    """Host-side prep: fold LN gains/scale into weights, build per-core maps."""
    f = lambda a: np.asarray(a, dtype=np.float32)
    x = np.asarray(inputs["x"]).astype(np.int64)
    tok_emb, pos_emb = f(inputs["tok_emb"]), f(inputs["pos_emb"])
    Wk, Wq, Wres = f(inputs["Wk"]), f(inputs["Wq"]), f(inputs["Wres"])
    ln1_g, ln1_b = f(inputs["ln1_g"]), f(inputs["ln1_b"])
    mlp_W, mlp_b = f(inputs["mlp_W"]), f(inputs["mlp_b"])
    ln2_g, ln2_b = f(inputs["ln2_g"]), f(inputs["ln2_b"])
    lnf_g, lnf_b = f(inputs["lnf_g"]), f(inputs["lnf_b"])
    proj_W, proj_b = f(inputs["proj_W"]), f(inputs["proj_b"])

    bf = lambda a: np.ascontiguousarray(a).astype(ml_dtypes.bfloat16)
    scale = np.float32(E) ** -0.5

    # [H,E,HD] -> [E, H*HD], fold ln gain (rows) and score scale into Wk
    wk_eff = np.stack([(Wk[i].transpose(1, 0, 2).reshape(E, E)
                        * ln1_g[i][:, None] * scale) for i in range(NB)])
    wq_eff = np.stack([(Wq[i].transpose(1, 0, 2).reshape(E, E)
                        * ln1_g[i][:, None]) for i in range(NB)])
    wres_eff = Wres[:NB]
    wmlp_eff = np.stack([mlp_W[i] * ln2_g[i][:, None] for i in range(NB)])
    mbias_eff = np.stack([ln2_b[i] @ mlp_W[i] + mlp_b[i] for i in range(NB)])
    projw_eff = proj_W * lnf_g[:, None]
    projb_eff = proj_b + lnf_b @ proj_W

    kbias = np.stack([ln1_b[i] @ (Wk[i].transpose(1, 0, 2).reshape(E, E) * scale)
                      for i in range(NB)])
    qbias = np.stack([ln1_b[i] @ Wq[i].transpose(1, 0, 2).reshape(E, E)
                      for i in range(NB)])
    with_kqbias = bool(np.any(kbias) or np.any(qbias))
    with_projbias = bool(np.any(projb_eff))

    mask01 = np.ascontiguousarray(np.tril(np.ones((128, 128), np.float32)).T)

    common = {
        "wk": bf(wk_eff), "wq": bf(wq_eff), "wres": bf(wres_eff),
        "wmlp": bf(wmlp_eff), "mbias": np.ascontiguousarray(mbias_eff),
        "mask01": bf(mask01),
    }
    if with_kqbias:
        common["kbias"] = bf(kbias)
        common["qbias"] = bf(qbias)

    in_maps = []
    for c in range(NCORES):
        b, jv = c // 2, c % 2
        emb = tok_emb[x[b]] + pos_emb[:T]
        m = dict(common)
        m["h0T"] = np.ascontiguousarray(emb.T.astype(np.float32))
        m["projw"] = bf(projw_eff[:, jv * VO:(jv + 1) * VO])
        if with_projbias:
            m["projb"] = bf(projb_eff[jv * VO:(jv + 1) * VO])
        in_maps.append(m)
    return in_maps, with_kqbias, with_projbias


_PROGRAM_CACHE = {}


def kernel(**inputs):
    VO = VOCAB // 2
    in_maps, with_kqbias, with_projbias = prepare_inputs(inputs, NB_FULL, VO)
    key = (NB_FULL, VO, with_kqbias, with_projbias)
    if key not in _PROGRAM_CACHE:
        _PROGRAM_CACHE[key] = build_program(NB_FULL, VO, with_kqbias, with_projbias)
    nc = _PROGRAM_CACHE[key]
    res = run_bass_kernel_spmd(nc, in_maps, list(range(NCORES))).results
    out = np.empty((B, T, VOCAB), np.float32)
    for c in range(NCORES):
        b, jv = c // 2, c % 2
        out[b, :, jv * VO:(jv + 1) * VO] = res[c]["logits"]
    return out
